# revision 12
# baseline (speedup 1.0000x reference)
"""Multi-headed self-attention (B=4, S=2048, D=1024, H=16) on 8 Trainium2 NeuronCores.

Sharding: core c -> batch b=c//2, head-group g=c%2 (8 heads = 512 output dims).
Each core computes out[b, :, g*512:(g+1)*512]; no collectives.

Per-core algorithm (bf16 matmuls, f32 accumulation):
  - host pre-transposes x[b].T and W[g].T slices to bf16
  - projections on PE: qT,kT [512,2048] (bias folded per-partition), v [2048,512]
    with a ones column appended per head (v_aug [t, 65])
  - per head: scores.T[t,s] = kT.T@qT (K=64); exp on ScalarE (scale=1/8 folded),
    N=2048 activations from PSUM; h.T_aug[65,s] += v_aug.T @ expS accumulated over t
    (row 64 = softmax denominator, for free)
  - h.T_aug -> bf16 -> DMA-xbar-transpose -> h_aug [s, 65]; normalize by 1/denom and
    add bv (valid since softmax rows sum to 1) on DVE; gather heads; DMA out.
"""
import sys

if "/opt/trn_rl_repo" not in sys.path:
    sys.path.insert(0, "/opt/trn_rl_repo")

import numpy as np
import ml_dtypes

B, S, D, H = 4, 2048, 1024, 16
N_CORES = 8
HEADS_PER_CORE = 8          # 8 heads x 64 dims = 512 output dims per core
DG = 512                    # per-core projection output dims
BF16 = ml_dtypes.bfloat16

_cached_nc = None


def _build_nc():
    import concourse.bass as bass  # noqa: F401
    import concourse.mybir as mybir
    import concourse.tile as tile
    from concourse import bacc

    AF = mybir.ActivationFunctionType
    f32 = mybir.dt.float32
    bf16 = mybir.dt.bfloat16

    nc = bacc.Bacc("TRN2", target_bir_lowering=False, debug=False,
                   num_devices=N_CORES)

    xT = nc.declare_dram_parameter("xT", [D, S], bf16, isOutput=False)
    wqT = nc.declare_dram_parameter("wqT", [D, DG], bf16, isOutput=False)
    wkT = nc.declare_dram_parameter("wkT", [D, DG], bf16, isOutput=False)
    wvT = nc.declare_dram_parameter("wvT", [D, DG], bf16, isOutput=False)
    bq = nc.declare_dram_parameter("bq", [DG], f32, isOutput=False)
    bk = nc.declare_dram_parameter("bk", [DG], f32, isOutput=False)
    bvb = nc.declare_dram_parameter("bvb", [128, DG], f32, isOutput=False)
    y = nc.declare_dram_parameter("y", [S, DG], f32, isOutput=True)

    NT = S // 128            # 16 t-chunks
    NSB = S // 512           # 4 s-blocks
    NJ = DG // 128           # 4 projection dim-chunks
    ND = D // 128            # 8 contraction chunks

    with tile.TileContext(nc) as tc:
        with (
            tc.tile_pool(name="const", bufs=1) as cpool,
            tc.tile_pool(name="qkv", bufs=1) as qkv,
            tc.tile_pool(name="expp", bufs=18) as expp,
            tc.tile_pool(name="hta", bufs=4) as htap,
            tc.tile_pool(name="htt", bufs=2) as http,
            tc.tile_pool(name="outp", bufs=1) as outp,
            tc.tile_pool(name="misc", bufs=4) as misc,
            tc.tile_pool(name="ht", bufs=2, space="PSUM") as htp,
            tc.tile_pool(name="sc", bufs=2, space="PSUM") as scp,
            tc.tile_pool(name="pj", bufs=2, space="PSUM") as pjp,
        ):
            # ---- input loads ----
            xt = cpool.tile([128, ND, S], bf16, tag="xt")
            xt_r = xT.rearrange("(i p) t -> p i t", p=128)
            for q in range(4):
                nc.sync.dma_start(xt[:, :, q * 512:(q + 1) * 512],
                                  xt_r[:, :, q * 512:(q + 1) * 512])
            wq_sb = cpool.tile([128, ND, DG], bf16, tag="wq")
            nc.sync.dma_start(wq_sb[:], wqT.rearrange("(i p) d -> p i d", p=128))
            wk_sb = cpool.tile([128, ND, DG], bf16, tag="wk")
            nc.sync.dma_start(wk_sb[:], wkT.rearrange("(i p) d -> p i d", p=128))
            wv_sb = cpool.tile([128, ND, DG], bf16, tag="wv")
            nc.sync.dma_start(wv_sb[:], wvT.rearrange("(i p) d -> p i d", p=128))
            bq_sb = cpool.tile([128, NJ], f32, tag="bq")
            nc.sync.dma_start(bq_sb[:], bq.rearrange("(j p) -> p j", p=128))
            bk_sb = cpool.tile([128, NJ], f32, tag="bk")
            nc.sync.dma_start(bk_sb[:], bk.rearrange("(j p) -> p j", p=128))
            bvb_sb = cpool.tile([128, DG], f32, tag="bvb")
            nc.sync.dma_start(bvb_sb[:], bvb[:])

            qt = qkv.tile([128, NJ, S], bf16, tag="qt")
            kt = qkv.tile([128, NJ, S], bf16, tag="kt")
            vaug = qkv.tile([128, NT, HEADS_PER_CORE * 65], bf16, tag="vaug")
            outsb = outp.tile([128, NT, DG], f32, tag="outsb")

            # ---- projections ----
            # ones columns for v_aug (col 64 of each 65-block survives)
            nc.gpsimd.memset(vaug[:], 1.0)

            def v_chain(half, tt):
                # v dims [half*256, half*256+256) for t-tile tt (heads 4*half..4*half+3)
                ps = pjp.tile([128, 256], f32, tag="pj", name=f"pv{half}_{tt}")
                d0 = half * 256
                with nc.named_scope("vproj"):
                    for i in range(ND):
                        nc.tensor.matmul(
                            ps[:], xt[:, i, tt * 128:(tt + 1) * 128],
                            wv_sb[:, i, d0:d0 + 256],
                            start=(i == 0), stop=(i == ND - 1))
                vv = vaug[:, tt, half * 260:(half + 1) * 260].rearrange(
                    "p (h w) -> p h w", w=65)
                pv = ps[:].rearrange("p (h w) -> p h w", w=64)
                nc.vector.tensor_copy(vv[:, :, 0:64], pv)

            def qk_chain(j, idx):
                # idx 0..7 -> (q/k, tb)
                which, tb = idx % 2, idx // 2
                w_sb, b_sb, dst = ((wq_sb, bq_sb, qt), (wk_sb, bk_sb, kt))[which]
                ps = pjp.tile([128, 512], f32, tag="pj", name=f"pqk{j}_{idx}")
                with nc.named_scope("qkproj"):
                    for i in range(ND):
                        nc.tensor.matmul(
                            ps[:], w_sb[:, i, j * 128:(j + 1) * 128],
                            xt[:, i, tb * 512:(tb + 1) * 512],
                            start=(i == 0), stop=(i == ND - 1))
                nc.vector.tensor_scalar_add(
                    dst[:, j, tb * 512:(tb + 1) * 512], ps[:], b_sb[:, j:j + 1])

            def finish_head(h, hta):
                # transpose h.T_aug -> [s, 65(+pad)], normalize + bias
                htt = http.tile([128, NT, 80], bf16, tag="htt", name=f"htt{h}")
                nc.sync.dma_start_transpose(htt[:], hta[:])
                rcp = misc.tile([128, NT], f32, tag="rcp", name=f"rcp{h}")
                nc.vector.reciprocal(rcp[:], htt[:, :, 64])
                o = outsb[:, :, h * 64:(h + 1) * 64]
                nc.vector.tensor_mul(
                    o, htt[:, :, 0:64],
                    rcp[:, :, None].broadcast_to([128, NT, 64]))
                nc.vector.tensor_add(
                    o, o,
                    bvb_sb[:, None, h * 64:(h + 1) * 64].broadcast_to(
                        [128, NT, 64]))
                nc.sync.dma_start(
                    y[:, h * 64:(h + 1) * 64].rearrange("(c p) w -> p c w", p=128),
                    outsb[:, :, h * 64:(h + 1) * 64])

            # deferred h.T-accumulation state: (j, sbo, ex_tiles) awaiting emission
            pending = []

            def emit_pending():
                if not pending:
                    return
                j, sbo, ex_tiles = pending.pop()
                hA, hB = 2 * j, 2 * j + 1
                accs = [htp.tile([65, 512], f32, tag="ht",
                                 name=f"ht{j}_{sbo}_{i}") for i in range(2)]
                with nc.named_scope("hT"):
                    for t in range(NT):
                        for i, h in enumerate((hA, hB)):
                            nc.tensor.matmul(
                                accs[i][:],
                                vaug[:, t, h * 65:h * 65 + 65],
                                ex_tiles[t][:, i * 512:(i + 1) * 512],
                                start=(t == 0), stop=(t == NT - 1))
                for i in range(2):
                    nc.vector.tensor_copy(
                        htas[(j, i)][0:65, sbo * 512:sbo * 512 + 512], accs[i][:])
                if sbo == NSB - 1:
                    finish_head(hA, htas.pop((j, 0)))
                    finish_head(hB, htas.pop((j, 1)))

            htas = {}

            def attn_pair(j):
                # heads A=2j (qt/kt partitions 0:64), B=2j+1 (64:128),
                # row-group-paired scores matmuls (K=64 each half of PE)
                for i, h in enumerate((2 * j, 2 * j + 1)):
                    hta = htap.tile([80, S], bf16, tag="hta", name=f"hta{h}")
                    nc.gpsimd.memset(hta[64:80, :], 0.0)
                    htas[(j, i)] = hta
                for sbo in range(NSB):          # s-blocks of 512
                    s0 = sbo * 512
                    ex_tiles = []
                    for t in range(NT):
                        sc = scp.tile([128, 1024], f32, tag="sc",
                                      name=f"sc{j}_{sbo}_{t}")
                        with nc.named_scope("scores"):
                            nc.tensor.matmul(
                                sc[:, 0:512],
                                kt[0:64, j, t * 128:(t + 1) * 128],
                                qt[0:64, j, s0:s0 + 512],
                                start=True, stop=True)
                            nc.tensor.matmul(
                                sc[:, 512:1024],
                                kt[64:128, j, t * 128:(t + 1) * 128],
                                qt[64:128, j, s0:s0 + 512],
                                start=True, stop=True)
                        ex = expp.tile([128, 1024], bf16, tag="expS",
                                       name=f"ex{j}_{sbo}_{t}")
                        nc.scalar.activation(ex[:], sc[:], AF.Exp, scale=0.125)
                        ex_tiles.append(ex)
                        # interleaved filler/deferred work, tuned for overlap
                        if t == 1:
                            emit_pending()
                        if sbo == 0 and j in (0, 2):
                            v_chain(j // 2, t)
                        if t in (5, 11) and j < NJ - 1:
                            qk_chain(j + 1, sbo * 2 + (1 if t == 11 else 0))
                    pending.append((j, sbo, ex_tiles))

            for idx in range(8):
                qk_chain(0, idx)
            for j in range(NJ):
                attn_pair(j)
            emit_pending()

    nc.compile()
    return nc


# results of the most recent device run (for test harnesses / profiling)
last_results = None


def kernel(x, Wq, bq, Wk, bk, Wv, bv):
    global _cached_nc, last_results
    from concourse.bass_utils import run_bass_kernel_spmd

    if _cached_nc is None:
        _cached_nc = _build_nc()
    nc = _cached_nc

    x = np.asarray(x, dtype=np.float32)
    xT = [np.ascontiguousarray(x[b].T).astype(BF16) for b in range(B)]
    wT = {}
    for name, W in (("q", Wq), ("k", Wk), ("v", Wv)):
        W = np.asarray(W, dtype=np.float32)
        for g in range(2):
            wT[(name, g)] = np.ascontiguousarray(
                W[g * DG:(g + 1) * DG, :].T).astype(BF16)
    bq = np.asarray(bq, dtype=np.float32)
    bk = np.asarray(bk, dtype=np.float32)
    bv = np.asarray(bv, dtype=np.float32)

    in_maps = []
    for c in range(N_CORES):
        b, g = c // 2, c % 2
        in_maps.append({
            "xT": xT[b],
            "wqT": wT[("q", g)], "wkT": wT[("k", g)], "wvT": wT[("v", g)],
            "bq": np.ascontiguousarray(bq[g * DG:(g + 1) * DG]),
            "bk": np.ascontiguousarray(bk[g * DG:(g + 1) * DG]),
            "bvb": np.ascontiguousarray(
                np.tile(bv[None, g * DG:(g + 1) * DG], (128, 1))),
        })

    last_results = run_bass_kernel_spmd(nc, in_maps, list(range(N_CORES)))

    out = np.empty((B, S, D), dtype=np.float32)
    for c in range(N_CORES):
        b, g = c // 2, c % 2
        out[b, :, g * DG:(g + 1) * DG] = last_results.results[c]["y"]
    return out
